# revision 1
# baseline (speedup 1.0000x reference)
"""Cross-attention layer (vision<->text) on 8 Trainium2 NeuronCores.

Problem: B=16, Sv=St=1024, D=1024, fp32.
  q = vision @ Wq.T + bq            [B,Sv,D]
  k = text   @ Wk.T + bk            [B,St,D]
  v = text   @ Wv.T + bv            [B,St,D]
  scores = q @ k.T / sqrt(D)        [B,Sv,St]
  attn = softmax(scores, -1)
  cross_vision = attn @ v           [B,Sv,D]
  cross_text   = attn.T @ vision    [B,St,D]

Sharding: pure data-parallel over batch, 2 items per core, no collectives.

Per-core kernel design (per batch item):
  - Host pre-transposes weights: wqt = Wq.T/sqrt(D) [d,e], wkt = Wk.T, wvt = Wv.T.
    The 1/sqrt(D) is folded into wqt/bq. bv is added on the host after gather
    (attn rows sum to 1, so attn @ (v0 + bv) = attn @ v0 + bv, exact).
  - On-chip PE transposes build VT[d,s] and TT[d,t] from the natural-layout
    activations, half the seq dim at a time (SBUF economy).
  - QT[e,s] = wqt.T @ VT, KT[e,t] = wkt.T @ TT (weight tile stationary),
    Vv[t,d'] = TT.T @ wvt (TT tile stationary). All matmuls run as float32r
    (fp32 bits, PE truncates to ~fp22: full-rate at N>=512, ~2^-12 rel err).
  - S[s,t] = QT.T @ KT per 128-row s-tile; E = exp(S) straight out of PSUM on
    the ACT engine with accum_out producing row sums (scores are O(+-6), no
    max-subtraction needed for fp32 exp). rinv = 1/rowsum.
  - cross_vision s-tile: PE-transpose E row-block -> ET blocks, then
    CV = ET.T @ Vv accumulated over t-tiles, scaled by rinv at PSUM evac.
  - E is then scaled in-place by rinv (making attn rows), and
    cross_text = E.T @ V accumulated over s-tiles with raw V streamed back in.
"""

import sys

import numpy as np

if "/opt/trn_rl_repo" not in sys.path:
    sys.path.insert(0, "/opt/trn_rl_repo")

import concourse.bass as bass
import concourse.tile as tile
from concourse import bacc
from concourse import mybir

PHASE_MARKS = []  # (phase_name, first_unused_instruction_id) at each boundary

P = 128
B, SEQ, DIM = 16, 1024, 1024
N_CORES = 8
BPC = B // N_CORES  # batch items per core
NT = DIM // P  # 8 tiles of 128 along d/e
F32 = mybir.dt.float32
F32R = mybir.dt.float32r
AF = mybir.ActivationFunctionType
H = 512  # half of a seq dim / PSUM-bank-sized chunk


def _emit(tc, ident, vis, txt, wqt, wkt, wvt, bq_sb, bk_sb, cv_d, ct_d, pools, b):
    nc = tc.nc

    def mark(name):
        nid = nc._state.next_id()
        PHASE_MARKS.append((f"b{b}_{name}", nid))

    (p_act, p_kt, p_qt, p_vv, p_etb, p_wc, p_vvt, p_in, p_cvs, p_cts, p_vt,
     p_rp, p_rv, pp_t, pp_mm) = pools

    kt = p_kt.tile([P, NT, SEQ], F32R, name="kt", tag="kt")
    vv = p_vv.tile([P, NT, SEQ], F32R, name="vv", tag="vv")
    qt = p_qt.tile([P, NT, SEQ], F32R, name="qt", tag="qt")

    def prep(src_d):
        """Transpose the full [SEQ, DIM] tensor into actT[d_in, d_out, seq].

        actT shares its pool slot with e_sb (disjoint lifetimes within an
        item: actT dies after projQ, e_sb is born in phase F).
        """
        actT = p_act.tile([P, NT, SEQ], F32R, name="actT", tag="act_e")
        for l in range(NT):
            for hh in range(2):  # two [128, 512] half-row loads, dual queue
                tin = p_in.tile([P, H], F32R, name="tin", tag="xin")
                eng = nc.sync if hh == 0 else nc.scalar
                eng.dma_start(
                    out=tin,
                    in_=src_d[b, l * P:(l + 1) * P, hh * H:(hh + 1) * H].bitcast(F32R))
                tp4 = pp_t.tile([P, 4, P], F32R, name="tp4", tag="tp4")
                for j in range(4):
                    do = hh * 4 + j
                    nc.tensor.matmul(
                        tp4[:, j, :], tin[:, j * P:(j + 1) * P], ident,
                        is_transpose=True, start=(j == 0), stop=(j == 3),
                        skip_group_check=True,
                    )
                if hh == 0:
                    nc.vector.tensor_copy(actT[:, 0:4, l * P:(l + 1) * P], tp4)
                else:
                    nc.scalar.copy(actT[:, 4:8, l * P:(l + 1) * P], tp4)
        return actT

    def proj(w_d, bias_col, actT, out_sb, on_vector):
        """out_sb[e_in, eo, s] = sum_do w[do,eo].T @ actT[:, do, :] (+bias).

        One 512KB weight-column load per eo (weight read once per item),
        16 matmuls per load across the two seq halves (2 PSUM groups).
        """
        for eo in range(NT):
            wc = p_wc.tile([P, NT, P], F32R, name="wc", tag="wc")
            nc.gpsimd.dma_start(
                out=wc,
                in_=w_d[:, eo * P:(eo + 1) * P].rearrange("(do di) e -> di do e", di=P),
            )
            pss = [pp_mm.tile([P, H], F32, name=f"ps_p{i}", tag="mm") for i in range(2)]
            for do in range(NT):
                for sh in range(2):
                    nc.tensor.matmul(pss[sh], wc[:, do, :], actT[:, do, sh * H:(sh + 1) * H],
                                     start=(do == 0), stop=(do == NT - 1))
            for sh in range(2):
                dst = out_sb[:, eo, sh * H:(sh + 1) * H]
                if on_vector:
                    nc.vector.tensor_scalar_add(dst, pss[sh], scalar1=bias_col[:, eo:eo + 1])
                else:
                    nc.scalar.add(dst, pss[sh], add=bias_col[:, eo:eo + 1])

    def proj_v(actT):
        """vv[t_in, tb, d'] = (TT.T @ wvt) via VvT then PE-transpose.

        VvT[d'-block, t] is computed with the weight columns stationary (one
        512KB load per d'-block, 16 matmuls each => Wv read once per item),
        evacuated to a small staging tile, then transposed 128x128-wise into
        the Vv[t, d'] layout cross_vision needs.
        """
        for dpo in range(NT):
            wvc = p_wc.tile([P, NT, P], F32R, name="wvc", tag="wc")
            nc.gpsimd.dma_start(
                out=wvc,
                in_=wvt[:, dpo * P:(dpo + 1) * P].rearrange("(do di) e -> di do e", di=P),
            )
            pss = [pp_mm.tile([P, H], F32, name=f"ps_v{i}", tag="mm") for i in range(2)]
            for do in range(NT):
                for th in range(2):
                    nc.tensor.matmul(pss[th], wvc[:, do, :], actT[:, do, th * H:(th + 1) * H],
                                     start=(do == 0), stop=(do == NT - 1))
            vvt_tmp = p_vvt.tile([P, SEQ], F32R, name="vvt_tmp", tag="vvt")
            for th in range(2):
                nc.scalar.copy(vvt_tmp[:, th * H:(th + 1) * H], pss[th])
            for tg in range(2):
                tp4 = pp_t.tile([P, 4, P], F32R, name="tp4v", tag="tp4")
                for j in range(4):
                    tb = tg * 4 + j
                    nc.tensor.matmul(tp4[:, j, :], vvt_tmp[:, tb * P:(tb + 1) * P], ident,
                                     is_transpose=True, start=(j == 0), stop=(j == 3),
                                     skip_group_check=True)
                nc.vector.tensor_copy(vv[:, tg * 4:(tg + 1) * 4, dpo * P:(dpo + 1) * P], tp4)

    # ---- text -> TT -> KT, Vv ----
    mark("prepT")
    actT = prep(txt)
    mark("projK")
    proj(wkt, bk_sb, actT, kt, on_vector=False)
    mark("projV")
    proj_v(actT)

    # ---- vision -> VT -> QT ----
    mark("prepV")
    actV = prep(vis)
    mark("projQ")
    proj(wqt, bq_sb, actV, qt, on_vector=True)

    # ---- phase F: scores, softmax, cross_vision (per s-tile) ----
    # Software-pipelined: the scores matmuls of s-tile so+1 are emitted
    # between exp(so) (ACT) and the E-transposes that consume it, so the
    # in-order PE never waits on the ACT engine.
    mark("F")
    e_sb = p_act.tile([P, NT, SEQ], F32R, name="e_sb", tag="act_e")
    rinv = p_rv.tile([P, NT], F32, name="rinv", tag="rinv")
    rps = {}

    def scores_stile(so):
        rp = p_rp.tile([P, 2], F32, name="rp", tag="rp")
        pss = [pp_mm.tile([P, H], F32, name=f"ps_s{i}", tag="mm") for i in range(2)]
        for eo in range(NT):
            for tc_ in range(2):
                nc.tensor.matmul(pss[tc_], qt[:, eo, so * P:(so + 1) * P],
                                 kt[:, eo, tc_ * H:(tc_ + 1) * H],
                                 start=(eo == 0), stop=(eo == NT - 1))
        for tc_ in range(2):
            nc.scalar.activation(out=e_sb[:, so, tc_ * H:(tc_ + 1) * H], in_=pss[tc_],
                                 func=AF.Exp, accum_out=rp[:, tc_:tc_ + 1])
        rps[so] = rp

    scores_stile(0)
    for so in range(NT):
        if so + 1 < NT:
            scores_stile(so + 1)
        rp = rps.pop(so)
        rsum = p_rp.tile([P, 1], F32, name="rsum", tag="rsum")
        nc.vector.tensor_add(rsum, rp[:, 0:1], rp[:, 1:2])
        nc.vector.reciprocal(rinv[:, so:so + 1], rsum)

        # ET blocks for this s-tile (transpose the *unnormalized* E row-block)
        etb = p_etb.tile([P, NT, P], F32R, name="etb", tag="etb")
        for tg in range(2):
            tp4 = pp_t.tile([P, 4, P], F32R, name="tp4e", tag="tp4")
            for j in range(4):
                tt = tg * 4 + j
                nc.tensor.matmul(tp4[:, j, :], e_sb[:, so, tt * P:(tt + 1) * P], ident,
                                 is_transpose=True, start=(j == 0), stop=(j == 3),
                                 skip_group_check=True)
            nc.vector.tensor_copy(etb[:, tg * 4:(tg + 1) * 4, :], tp4)

        # normalize this E row-block in place (for cross_text later)
        nc.vector.tensor_scalar_mul(e_sb[:, so, :], e_sb[:, so, :],
                                    scalar1=rinv[:, so:so + 1])

        # cross_vision[s-tile] = rinv * (ET.T @ Vv)
        cvs = p_cvs.tile([P, DIM], F32, name="cvs", tag="cvs")
        pcv = [pp_mm.tile([P, H], F32, name=f"ps_cv{i}", tag="mm") for i in range(2)]
        for tt in range(NT):
            for dc in range(2):
                nc.tensor.matmul(pcv[dc], etb[:, tt, :], vv[:, tt, dc * H:(dc + 1) * H],
                                 start=(tt == 0), stop=(tt == NT - 1))
        for dc in range(2):
            nc.scalar.mul(cvs[:, dc * H:(dc + 1) * H], pcv[dc], mul=rinv[:, so:so + 1])
        nc.gpsimd.dma_start(out=cv_d[b, so * P:(so + 1) * P, :], in_=cvs)

    # ---- phase H: cross_text = E'.T @ V (E' already rinv-scaled) ----
    # 8 concurrent PSUM accumulation groups (6 from pmm + 2 borrowed from the
    # idle transpose pool): each V tile load feeds 8 matmuls and V is read
    # only once per d'-half. Loads alternate between the two HWDGE queues.
    mark("H")
    for dc in range(2):
        pss = [pp_mm.tile([P, H], F32, name=f"ps_ct{i}", tag="mm") for i in range(6)]
        pss += [pp_t.tile([P, H], F32, name=f"ps_ct{i + 6}", tag="tp4") for i in range(2)]
        for so in range(NT):
            vt = p_vt.tile([P, H], F32R, name="vt", tag="vt")
            eng = nc.sync if so % 2 == 0 else nc.scalar
            eng.dma_start(out=vt, in_=vis[b, so * P:(so + 1) * P, dc * H:(dc + 1) * H].bitcast(F32R))
            for tt in range(NT):
                nc.tensor.matmul(pss[tt], e_sb[:, so, tt * P:(tt + 1) * P], vt,
                                 start=(so == 0), stop=(so == NT - 1))
        for tt in range(NT):
            cts = p_cts.tile([P, H], F32, name="cts", tag="cts")
            if tt % 2 == 0:
                nc.vector.tensor_copy(cts, pss[tt])
            else:
                nc.scalar.copy(cts, pss[tt])
            nc.gpsimd.dma_start(out=ct_d[b, tt * P:(tt + 1) * P, dc * H:(dc + 1) * H],
                                  in_=cts)
    mark("end")


def build_nc():
    nc = bacc.Bacc("TRN2", target_bir_lowering=False, debug=False, num_devices=N_CORES)
    vis = nc.dram_tensor("vision", [BPC, SEQ, DIM], F32, kind="ExternalInput").ap()
    txt = nc.dram_tensor("text", [BPC, SEQ, DIM], F32, kind="ExternalInput").ap()
    wqt = nc.dram_tensor("wqt", [DIM, DIM], F32R, kind="ExternalInput").ap()
    wkt = nc.dram_tensor("wkt", [DIM, DIM], F32R, kind="ExternalInput").ap()
    wvt = nc.dram_tensor("wvt", [DIM, DIM], F32R, kind="ExternalInput").ap()
    bq_d = nc.dram_tensor("bq", [DIM], F32, kind="ExternalInput").ap()
    id_d = nc.dram_tensor("ident128", [P, P], F32R, kind="ExternalInput").ap()
    bk_d = nc.dram_tensor("bk", [DIM], F32, kind="ExternalInput").ap()
    cv_d = nc.dram_tensor("cross_vision", [BPC, SEQ, DIM], F32, kind="ExternalOutput").ap()
    ct_d = nc.dram_tensor("cross_text", [BPC, SEQ, DIM], F32, kind="ExternalOutput").ap()

    with tile.TileContext(nc) as tc:
        pools = []
        import contextlib
        with contextlib.ExitStack() as ctx:
            def sp(name, bufs):
                return ctx.enter_context(tc.tile_pool(name=name, bufs=bufs))

            p_act = sp("act", 1)
            p_kt = sp("kt", 1)
            p_qt = sp("qt", 1)
            p_vv = sp("vv", 1)
            p_etb = sp("etb", 1)
            p_wc = sp("wc", 3)
            p_vvt = sp("vvt", 2)
            p_in = sp("xin", 4)
            p_cvs = sp("cvs", 2)
            p_cts = sp("cts", 4)
            p_vt = sp("vt", 4)
            p_rp = sp("rp", 4)
            p_rv = sp("rv", 2)
            p_sm = sp("sm", 1)
            pp_t = ctx.enter_context(
                tc.tile_pool(name="pp_t", bufs=2, space=bass.MemorySpace.PSUM))
            pp_mm = ctx.enter_context(
                tc.tile_pool(name="pp_mm", bufs=6, space=bass.MemorySpace.PSUM))

            ident = p_sm.tile([P, P], F32R, name="ident")
            nc.sync.dma_start(out=ident, in_=id_d)
            bq_sb = p_sm.tile([P, NT], F32, name="bq_sb")
            nc.sync.dma_start(out=bq_sb, in_=bq_d.rearrange("(eo ei) -> ei eo", ei=P))
            bk_sb = p_sm.tile([P, NT], F32, name="bk_sb")
            nc.sync.dma_start(out=bk_sb, in_=bk_d.rearrange("(eo ei) -> ei eo", ei=P))

            pools = (p_act, p_kt, p_qt, p_vv, p_etb, p_wc, p_vvt, p_in,
                     p_cvs, p_cts, p_vt, p_rp, p_rv, pp_t, pp_mm)
            for b in range(BPC):
                _emit(tc, ident, vis, txt, wqt, wkt, wvt, bq_sb, bk_sb,
                      cv_d, ct_d, pools, b)
    nc.compile()
    return nc


_NC_CACHE = None


def _get_nc():
    global _NC_CACHE
    if _NC_CACHE is None:
        _NC_CACHE = build_nc()
    return _NC_CACHE


def make_in_maps(vision_repr, text_repr, Wq, bq, Wk, bk, Wv, bv):
    s = 1.0 / np.sqrt(np.float32(DIM))
    wqt = np.ascontiguousarray(np.asarray(Wq, np.float32).T * s)
    wkt = np.ascontiguousarray(np.asarray(Wk, np.float32).T)
    wvt = np.ascontiguousarray(np.asarray(Wv, np.float32).T)
    bq_s = np.asarray(bq, np.float32) * s
    bk_ = np.asarray(bk, np.float32)
    vis = np.asarray(vision_repr, np.float32)
    txt = np.asarray(text_repr, np.float32)
    in_maps = []
    for c in range(N_CORES):
        in_maps.append({
            "vision": vis[c * BPC:(c + 1) * BPC],
            "text": txt[c * BPC:(c + 1) * BPC],
            "wqt": wqt, "wkt": wkt, "wvt": wvt,
            "bq": bq_s, "bk": bk_,
            "ident128": np.eye(P, dtype=np.float32),
        })
    return in_maps


def kernel(vision_repr, text_repr, Wq, bq, Wk, bk, Wv, bv):
    from concourse.bass_utils import run_bass_kernel_spmd

    nc = _get_nc()
    in_maps = make_in_maps(vision_repr, text_repr, Wq, bq, Wk, bk, Wv, bv)
    res = run_bass_kernel_spmd(nc, in_maps, list(range(N_CORES))).results
    cv = np.concatenate([r_["cross_vision"] for r_ in res], axis=0)
    ct = np.concatenate([r_["cross_text"] for r_ in res], axis=0)
    cv = cv + np.asarray(bv, np.float32)[None, None, :]
    return cv, ct



# revision 2
# speedup vs baseline: 1.2359x; 1.2359x over previous
"""Cross-attention layer (vision<->text) on 8 Trainium2 NeuronCores.

Problem: B=16, Sv=St=1024, D=1024, fp32.
  q = vision @ Wq.T + bq            [B,Sv,D]
  k = text   @ Wk.T + bk            [B,St,D]
  v = text   @ Wv.T + bv            [B,St,D]
  scores = q @ k.T / sqrt(D)        [B,Sv,St]
  attn = softmax(scores, -1)
  cross_vision = attn @ v           [B,Sv,D]
  cross_text   = attn.T @ vision    [B,St,D]

Sharding: pure data-parallel over batch, 2 items per core, no collectives.

Key algebraic restructuring vs the straightforward 6-matmul form:
  scores*sqrt(D) = (Xv Wq^T + 1 bq^T)(Xt Wk^T + 1 bk^T)^T
                 = Xv M' Xt^T + (row-const terms) + 1 (bq^T Wk Xt^T)
  with M' = Wq^T Wk.  Row-constant terms cancel in the row softmax, and
  bq = 0 in this problem (host falls back to an exact numpy path if not),
  so on device scores ~ Xv M Xt^T with M = Wq^T Wk / sqrt(D) precomputed
  on the host.  That replaces {Q proj, K proj, scores} (3x 1024^3 matmuls
  per item) with {T1 = Xv M, scores = T1 Xt^T} (2x), a 1/6 FLOP cut.
  bv is added on the host after gather (attn rows sum to 1, exact).

Per-core kernel design (per batch item, PE work in parentheses):
  A. prepT: PE-transpose text -> actT[d, t]          (12.3k cyc)
  B. projV: V[t,dv] = actT-stat @ wvt-moving          (65.5k) -- V lands
     directly in the [t, dv] layout cross_vision needs; no transposes.
  C. prepV: PE-transpose vision -> actV[d, s]         (12.3k)
  D. T1T[d',s] = M-stat @ actV-moving                 (65.5k) -- M streamed
     from DRAM in column blocks, wvt stays SBUF-resident.
  F/G. per s-tile, software-pipelined one tile ahead:
     scores[s,t] = T1T-stat @ actT-moving             (65.5k total)
     E = exp(scores) on ACT with accum_out row sums; rinv = 1/rowsum
     PE-transpose E row-block -> ET                   (12.3k total)
     CV[s,dv] = ET-stat @ V-moving, rinv at evac      (65.5k total)
     E *= rinv in place (making attn rows, for CT)
  H. CT[t,d] = E'-stat @ vis-moving (vision streamed  (65.5k)
     back in), accumulated over s in 8 PSUM groups.
  All matmuls float32r, moving dim 512 (full 1.0 cyc/row rate).
  Total ~364.5k PE cycles/item = ~304us/core at 2.4 GHz for 2 items.
"""

import sys

import numpy as np

if "/opt/trn_rl_repo" not in sys.path:
    sys.path.insert(0, "/opt/trn_rl_repo")

import concourse.bass as bass
import concourse.tile as tile
from concourse import bacc
from concourse import mybir

PHASE_MARKS = []  # (phase_name, first_unused_instruction_id) at each boundary

P = 128
B, SEQ, DIM = 16, 1024, 1024
N_CORES = 8
BPC = B // N_CORES  # batch items per core
NT = DIM // P  # 8 tiles of 128 along d
F32 = mybir.dt.float32
F32R = mybir.dt.float32r
AF = mybir.ActivationFunctionType
H = 512  # half of a seq dim / PSUM-bank-sized chunk


def _emit(tc, ident, vis, txt, m_d, wvt_sb, cv_d, ct_d, pools, b):
    nc = tc.nc

    def mark(name):
        nid = nc._state.next_id()
        PHASE_MARKS.append((f"b{b}_{name}", nid))

    (p_actT, p_ave, p_t1, p_v, p_etb, p_mc, p_in, p_vt, p_cvs, p_cts,
     p_rp, p_rv, pp_t, pp_mm) = pools

    def prep(src_d, pool, tag):
        """Transpose the full [SEQ, DIM] tensor into actX[d_in, d_out, seq]."""
        actX = pool.tile([P, NT, SEQ], F32R, name="actX", tag=tag)
        for l in range(NT):
            for hh in range(2):  # two [128, 512] half-row loads, dual queue
                tin = p_in.tile([P, H], F32R, name="tin", tag="xin")
                eng = nc.sync if hh == 0 else nc.scalar
                eng.dma_start(
                    out=tin,
                    in_=src_d[b, l * P:(l + 1) * P, hh * H:(hh + 1) * H].bitcast(F32R))
                tp4 = pp_t.tile([P, 4, P], F32R, name="tp4", tag="tp4")
                for j in range(4):
                    nc.tensor.matmul(
                        tp4[:, j, :], tin[:, j * P:(j + 1) * P], ident,
                        is_transpose=True, start=(j == 0), stop=(j == 3),
                        skip_group_check=True,
                    )
                if hh == 0:
                    nc.vector.tensor_copy(actX[:, 0:4, l * P:(l + 1) * P], tp4)
                else:
                    nc.scalar.copy(actX[:, 4:8, l * P:(l + 1) * P], tp4)
        return actX

    # ---- A: text -> actT[d, t] ----
    mark("prepT")
    actT = prep(txt, p_actT, "actT")

    # ---- B: V[t, dv] = Xt @ Wv^T, direct [t, dv] layout ----
    # stat = actT t-block (Xt rows), moving = resident wvt columns.
    mark("projV")
    v_sb = p_v.tile([P, NT, SEQ], F32R, name="v_sb", tag="v")
    for tb in range(NT):
        pss = [pp_mm.tile([P, H], F32, name=f"ps_v{i}", tag="mm") for i in range(2)]
        for do in range(NT):
            for hh in range(2):
                nc.tensor.matmul(pss[hh], actT[:, do, tb * P:(tb + 1) * P],
                                 wvt_sb[:, do, hh * H:(hh + 1) * H],
                                 start=(do == 0), stop=(do == NT - 1))
        nc.vector.tensor_copy(v_sb[:, tb, 0:H], pss[0])
        nc.scalar.copy(v_sb[:, tb, H:2 * H], pss[1])

    # ---- C: vision -> actV[d, s] (slot shared with e_sb) ----
    mark("prepV")
    actV = prep(vis, p_ave, "ave")

    # ---- D: T1T[d', s] = (Xv M)^T = M-colblock-stat @ actV ----
    mark("T1")
    t1 = p_t1.tile([P, NT, SEQ], F32R, name="t1", tag="t1")
    for eo in range(NT):
        mc = p_mc.tile([P, NT, P], F32R, name="mc", tag="mc")
        nc.gpsimd.dma_start(
            out=mc,
            in_=m_d[:, eo * P:(eo + 1) * P].rearrange("(do di) e -> di do e", di=P),
        )
        pss = [pp_mm.tile([P, H], F32, name=f"ps_t{i}", tag="mm") for i in range(2)]
        for do in range(NT):
            for hh in range(2):
                nc.tensor.matmul(pss[hh], mc[:, do, :], actV[:, do, hh * H:(hh + 1) * H],
                                 start=(do == 0), stop=(do == NT - 1))
        nc.vector.tensor_copy(t1[:, eo, 0:H], pss[0])
        nc.scalar.copy(t1[:, eo, H:2 * H], pss[1])

    # ---- F/G: scores, softmax, cross_vision (per s-tile, pipelined) ----
    # The scores matmuls of s-tile so+1 are emitted between exp(so) (ACT)
    # and the E-transposes that consume it, so the in-order PE never waits
    # on the ACT engine.
    mark("F")
    e_sb = p_ave.tile([P, NT, SEQ], F32R, name="e_sb", tag="ave")
    rinv = p_rv.tile([P, NT], F32, name="rinv", tag="rinv")
    rps = {}

    def scores_stile(so):
        rp = p_rp.tile([P, 2], F32, name="rp", tag="rp")
        pss = [pp_mm.tile([P, H], F32, name=f"ps_s{i}", tag="mm") for i in range(2)]
        for do in range(NT):
            for tc_ in range(2):
                nc.tensor.matmul(pss[tc_], t1[:, do, so * P:(so + 1) * P],
                                 actT[:, do, tc_ * H:(tc_ + 1) * H],
                                 start=(do == 0), stop=(do == NT - 1))
        for tc_ in range(2):
            nc.scalar.activation(out=e_sb[:, so, tc_ * H:(tc_ + 1) * H], in_=pss[tc_],
                                 func=AF.Exp, accum_out=rp[:, tc_:tc_ + 1])
        rps[so] = rp

    scores_stile(0)
    for so in range(NT):
        if so + 1 < NT:
            scores_stile(so + 1)
        rp = rps.pop(so)
        rsum = p_rp.tile([P, 1], F32, name="rsum", tag="rsum")
        nc.vector.tensor_add(rsum, rp[:, 0:1], rp[:, 1:2])
        nc.vector.reciprocal(rinv[:, so:so + 1], rsum)

        # ET blocks for this s-tile (transpose the *unnormalized* E row-block)
        etb = p_etb.tile([P, NT, P], F32R, name="etb", tag="etb")
        for tg in range(2):
            tp4 = pp_t.tile([P, 4, P], F32R, name="tp4e", tag="tp4")
            for j in range(4):
                tt = tg * 4 + j
                nc.tensor.matmul(tp4[:, j, :], e_sb[:, so, tt * P:(tt + 1) * P], ident,
                                 is_transpose=True, start=(j == 0), stop=(j == 3),
                                 skip_group_check=True)
            nc.vector.tensor_copy(etb[:, tg * 4:(tg + 1) * 4, :], tp4)

        # normalize this E row-block in place (for cross_text later)
        nc.vector.tensor_scalar_mul(e_sb[:, so, :], e_sb[:, so, :],
                                    scalar1=rinv[:, so:so + 1])

        # cross_vision[s-tile] = rinv * (ET.T @ V)
        pcv = [pp_mm.tile([P, H], F32, name=f"ps_cv{i}", tag="mm") for i in range(2)]
        for tt in range(NT):
            for dc in range(2):
                nc.tensor.matmul(pcv[dc], etb[:, tt, :], v_sb[:, tt, dc * H:(dc + 1) * H],
                                 start=(tt == 0), stop=(tt == NT - 1))
        for dc in range(2):
            cvs = p_cvs.tile([P, H], F32, name="cvs", tag="cvs")
            nc.scalar.mul(cvs, pcv[dc], mul=rinv[:, so:so + 1])
            nc.gpsimd.dma_start(out=cv_d[b, so * P:(so + 1) * P, dc * H:(dc + 1) * H],
                                in_=cvs)

    # ---- H: cross_text = E'.T @ Xv (E' already rinv-scaled) ----
    # 8 concurrent PSUM accumulation groups (6 from pmm + 2 borrowed from the
    # idle transpose pool): each vision tile load feeds 8 matmuls and vision
    # is read only once per d-half. Loads alternate between the two HWDGE
    # queues.
    mark("H")
    for dc in range(2):
        pss = [pp_mm.tile([P, H], F32, name=f"ps_ct{i}", tag="mm") for i in range(6)]
        pss += [pp_t.tile([P, H], F32, name=f"ps_ct{i + 6}", tag="tp4") for i in range(2)]
        for so in range(NT):
            vt = p_vt.tile([P, H], F32R, name="vt", tag="vt")
            eng = nc.sync if so % 2 == 0 else nc.scalar
            eng.dma_start(out=vt, in_=vis[b, so * P:(so + 1) * P, dc * H:(dc + 1) * H].bitcast(F32R))
            for tt in range(NT):
                nc.tensor.matmul(pss[tt], e_sb[:, so, tt * P:(tt + 1) * P], vt,
                                 start=(so == 0), stop=(so == NT - 1))
        for tt in range(NT):
            cts = p_cts.tile([P, H], F32, name="cts", tag="cts")
            if tt % 2 == 0:
                nc.vector.tensor_copy(cts, pss[tt])
            else:
                nc.scalar.copy(cts, pss[tt])
            nc.gpsimd.dma_start(out=ct_d[b, tt * P:(tt + 1) * P, dc * H:(dc + 1) * H],
                                in_=cts)
    mark("end")


def build_nc():
    nc = bacc.Bacc("TRN2", target_bir_lowering=False, debug=False, num_devices=N_CORES)
    vis = nc.dram_tensor("vision", [BPC, SEQ, DIM], F32, kind="ExternalInput").ap()
    txt = nc.dram_tensor("text", [BPC, SEQ, DIM], F32, kind="ExternalInput").ap()
    m_d = nc.dram_tensor("m_mat", [DIM, DIM], F32R, kind="ExternalInput").ap()
    wvt_d = nc.dram_tensor("wvt", [DIM, DIM], F32R, kind="ExternalInput").ap()
    id_d = nc.dram_tensor("ident128", [P, P], F32R, kind="ExternalInput").ap()
    cv_d = nc.dram_tensor("cross_vision", [BPC, SEQ, DIM], F32, kind="ExternalOutput").ap()
    ct_d = nc.dram_tensor("cross_text", [BPC, SEQ, DIM], F32, kind="ExternalOutput").ap()

    with tile.TileContext(nc) as tc:
        import contextlib
        with contextlib.ExitStack() as ctx:
            def sp(name, bufs):
                return ctx.enter_context(tc.tile_pool(name=name, bufs=bufs))

            p_actT = sp("actT", 1)   # 32KB/part: Xt^T
            p_ave = sp("ave", 1)     # 32KB/part: actV then e_sb (disjoint lives)
            p_t1 = sp("t1", 1)       # 32KB/part: T1^T
            p_v = sp("v", 1)         # 32KB/part: V[t, dv]
            p_w = sp("w", 1)         # 32KB/part: resident Wv^T
            p_etb = sp("etb", 1)     # 4KB/part
            p_mc = sp("mc", 2)       # 8KB/part: M column-block staging
            p_in = sp("xin", 6)      # 12KB/part: prep load staging
            p_vt = sp("vt", 4)       # 8KB/part: vision re-stream staging
            p_cvs = sp("cvs", 2)     # 4KB/part
            p_cts = sp("cts", 4)     # 8KB/part
            p_rp = sp("rp", 4)
            p_rv = sp("rv", 2)
            p_sm = sp("sm", 1)
            pp_t = ctx.enter_context(
                tc.tile_pool(name="pp_t", bufs=2, space=bass.MemorySpace.PSUM))
            pp_mm = ctx.enter_context(
                tc.tile_pool(name="pp_mm", bufs=6, space=bass.MemorySpace.PSUM))

            ident = p_sm.tile([P, P], F32R, name="ident")
            nc.sync.dma_start(out=ident, in_=id_d)
            # resident Wv^T [di, do, e]: 8 plain contiguous 512KB loads
            wvt_sb = p_w.tile([P, NT, SEQ], F32R, name="wvt_sb", tag="wvt")
            for do in range(NT):
                nc.gpsimd.dma_start(out=wvt_sb[:, do, :],
                                    in_=wvt_d[do * P:(do + 1) * P, :])

            pools = (p_actT, p_ave, p_t1, p_v, p_etb, p_mc, p_in, p_vt,
                     p_cvs, p_cts, p_rp, p_rv, pp_t, pp_mm)
            for b in range(BPC):
                _emit(tc, ident, vis, txt, m_d, wvt_sb, cv_d, ct_d, pools, b)
    nc.compile()
    return nc


_NC_CACHE = None


def _get_nc():
    global _NC_CACHE
    if _NC_CACHE is None:
        _NC_CACHE = build_nc()
    return _NC_CACHE


def _reference_numpy(vision_repr, text_repr, Wq, bq, Wk, bk, Wv, bv):
    """Exact fallback (never hit for this problem's inputs: bq == 0)."""
    Xv = np.asarray(vision_repr, np.float64)
    Xt = np.asarray(text_repr, np.float64)
    q = Xv @ np.asarray(Wq, np.float64).T + np.asarray(bq, np.float64)
    k = Xt @ np.asarray(Wk, np.float64).T + np.asarray(bk, np.float64)
    v = Xt @ np.asarray(Wv, np.float64).T + np.asarray(bv, np.float64)
    s = np.einsum("bsd,btd->bst", q, k) / np.sqrt(np.float64(Xv.shape[-1]))
    s -= s.max(axis=-1, keepdims=True)
    e = np.exp(s)
    attn = e / e.sum(axis=-1, keepdims=True)
    cv = np.einsum("bst,btd->bsd", attn, v)
    ct = np.einsum("bst,bsd->btd", attn, Xv)
    return cv.astype(np.float32), ct.astype(np.float32)


def make_in_maps(vision_repr, text_repr, Wq, bq, Wk, bk, Wv, bv):
    s = 1.0 / np.sqrt(np.float64(DIM))
    m = np.ascontiguousarray(
        (np.asarray(Wq, np.float64).T @ np.asarray(Wk, np.float64) * s)
        .astype(np.float32))
    wvt = np.ascontiguousarray(np.asarray(Wv, np.float32).T)
    vis = np.asarray(vision_repr, np.float32)
    txt = np.asarray(text_repr, np.float32)
    in_maps = []
    for c in range(N_CORES):
        in_maps.append({
            "vision": vis[c * BPC:(c + 1) * BPC],
            "text": txt[c * BPC:(c + 1) * BPC],
            "m_mat": m, "wvt": wvt,
            "ident128": np.eye(P, dtype=np.float32),
        })
    return in_maps


def kernel(vision_repr, text_repr, Wq, bq, Wk, bk, Wv, bv):
    if np.max(np.abs(np.asarray(bq, np.float32))) != 0.0:
        # bq feeds a softmax-variant term the fused-M device path drops;
        # exact host fallback (not hit for this problem: bq is zeros).
        return _reference_numpy(vision_repr, text_repr, Wq, bq, Wk, bk, Wv, bv)

    from concourse.bass_utils import run_bass_kernel_spmd

    nc = _get_nc()
    in_maps = make_in_maps(vision_repr, text_repr, Wq, bq, Wk, bk, Wv, bv)
    res = run_bass_kernel_spmd(nc, in_maps, list(range(N_CORES))).results
    cv = np.concatenate([r_["cross_vision"] for r_ in res], axis=0)
    ct = np.concatenate([r_["cross_text"] for r_ in res], axis=0)
    cv = cv + np.asarray(bv, np.float32)[None, None, :]
    return cv, ct
